# revision 20
# baseline (speedup 1.0000x reference)
"""GCN 3-layer kernel for Trainium2, 8-core SPMD.

Math (per layer, PyG GCN convention with self-loops, factorized):
    deg[d]  = indegree(d) + 1;  dinv = deg^-1/2
    y       = dinv[:,None] * (h @ W)                    (per-node scale)
    agg[d]  = sum_{e: dst[e]=d} y[src[e]]  + y[d]       (self-loop direct)
    h_next  = dinv[:,None] * agg + b                    (+ relu on last layer)

Distribution: destination-sharded across 8 cores (6272 = 49*128 node slots
per core, padded to 50176 total).  Each core computes y for its own nodes;
TWO AllGathers (split by node-position within the shard: windows 0-31 "a",
windows 32-48 "b") replicate the y table to every core's DRAM so that
"a"-stream gathers can start while the "b" collective is still in flight.
Layer l+1's phase A is interleaved into layer l's phase B window-by-window
so the collectives fire as early as possible.  Each core gathers message
rows with dma_gather (4 SWDGE queues, greedy-balanced) and scatter-adds
them with one-hot matmuls on the PE (PSUM accumulation per 128-dst
window).

Key layout choices:
  * h state is kept feature-major [128 F, nodes].  Layers 1-2 aggregate in
    "aggT" orientation -- matmul(out=aggT[F, dst], lhsT=m[msg, F],
    rhs=S[msg, dst]) -- so the result lands feature-major with no PE
    transpose.  Layer 3 aggregates dst-major so the [N, 64] output can be
    DMA'd directly.
  * The one-hot S matrices for a whole window are built with a single wide
    tensor_tensor is_equal using stride-0 broadcast APs (iota bcast over
    blocks, dst-local bcast over the 128 compare lanes).
  * Self-loop contribution enters the PSUM chain as one identity matmul of
    the core's own y window (no gathered self-edges).

Split tables: y_full_a holds locals [0, 4096) of every core (32768 rows,
exactly the int16 limit), y_full_b locals [4096, 6272) (17408 rows); both
fit int16 gather indices directly (idx_a = 4096*core + local, idx_b =
2176*core + local - 4096).
"""

import numpy as np
import ml_dtypes

N_NODES = 50000
N_CORES = 8
PER_CORE = 6272            # 49 * 128
N_PAD = PER_CORE * N_CORES # 50176
N_WIN = PER_CORE // 128    # 49
WIN_A = 32                 # windows in the "a" half (a-table = 32768 rows, int16 limit)
LOC_A = WIN_A * 128        # 3200 locals in "a"
LOC_B = PER_CORE - LOC_A   # 3072 locals in "b"
ROWS_A = LOC_A * N_CORES   # 25600
ROWS_B = LOC_B * N_CORES   # 24576
F = 128                    # feature width (layer3 padded 64->128)
F_OUT = 64
GROUP_WINDOWS = 3          # windows per gather chunk
PREFETCH_A = 5             # groups of "a"-half gathers to emit ahead
PREFETCH_B = 1             # groups of "b"-half gathers to emit ahead

BF16 = ml_dtypes.bfloat16


def _wrap_idx16(idx: np.ndarray) -> np.ndarray:
    """Wrap a flat int16 index stream into the [128, n/16] layout dma_gather
    expects (element i at [i%16, i//16], replicated across the 8 groups of
    16 partitions)."""
    n = len(idx)
    assert n % 128 == 0
    cols = n // 16
    out = np.empty((128, cols), np.int16)
    w = idx.reshape(cols, 16).T  # [16, cols]
    for g in range(8):
        out[g * 16:(g + 1) * 16, :] = w
    return out


def _preprocess(edge_index: np.ndarray):
    """Host-side graph prep: degree norm, dst-sharding, per-window edge
    streams (a/b by source-local half), block padding shared across cores."""
    src = edge_index[0].astype(np.int64)
    dst = edge_index[1].astype(np.int64)
    deg = np.bincount(dst, minlength=N_NODES).astype(np.float64) + 1.0
    dinv = (1.0 / np.sqrt(deg)).astype(np.float32)
    dinv_pad = np.ones(N_PAD, np.float32)
    dinv_pad[:N_NODES] = dinv

    core_of = dst // PER_CORE
    win_of = (dst % PER_CORE) // 128
    dloc_of = dst % 128
    s_core = src // PER_CORE
    s_loc = src % PER_CORE
    is_a = s_loc < LOC_A
    idx_val = np.where(is_a, s_core * LOC_A + s_loc,
                       s_core * LOC_B + (s_loc - LOC_A))

    # sort once by (core, window)
    order = np.lexsort((dst, win_of, core_of))
    idx_s, core_s, win_s, dloc_s, a_s = (
        idx_val[order], core_of[order], win_of[order], dloc_of[order],
        is_a[order])

    # per (core, window, stream) counts
    counts = np.zeros((N_CORES, N_WIN, 2), np.int64)
    np.add.at(counts, (core_s, win_s, (~a_s).astype(np.int64)), 1)
    # shared block counts per window (max over cores)
    blk_a = -(-counts[:, :, 0].max(axis=0) // 128)  # [N_WIN]
    blk_b = -(-counts[:, :, 1].max(axis=0) // 128)  # [N_WIN]
    nblk = blk_a + blk_b

    # slot offsets within each stream
    off_a = np.concatenate([[0], np.cumsum(blk_a * 128)])
    off_b = np.concatenate([[0], np.cumsum(blk_b * 128)])
    gboff = np.concatenate([[0], np.cumsum(nblk)])  # global block offset/window
    n_a, n_b = int(off_a[-1]), int(off_b[-1])
    tot_blk = int(gboff[-1])

    # fill per-core padded streams; merged dl stream ordered
    # (window, a blocks, b blocks)
    idx_a = np.zeros((N_CORES, n_a), np.int16)
    idx_b = np.zeros((N_CORES, max(1, n_b)), np.int16)
    dl_all = np.full((N_CORES, tot_blk * 128), 999.0, np.float32)

    # boundaries of (core, window) groups in the sorted arrays
    keys = core_s * N_WIN + win_s
    bounds = np.searchsorted(keys, np.arange(N_CORES * N_WIN + 1))
    for c in range(N_CORES):
        for w in range(N_WIN):
            k = c * N_WIN + w
            sl = slice(bounds[k], bounds[k + 1])
            s_idx = idx_s[sl]; s_dl = dloc_s[sl]; s_a = a_s[sl]
            a_idx = s_idx[s_a]; a_dl = s_dl[s_a]
            b_idx = s_idx[~s_a]; b_dl = s_dl[~s_a]
            o = off_a[w]
            idx_a[c, o:o + len(a_idx)] = a_idx.astype(np.int16)
            o = off_b[w]
            idx_b[c, o:o + len(b_idx)] = b_idx.astype(np.int16)
            gb = gboff[w] * 128
            dl_all[c, gb:gb + len(a_dl)] = a_dl
            gb2 = gboff[w] * 128 + int(blk_a[w]) * 128
            dl_all[c, gb2:gb2 + len(b_dl)] = b_dl

    return dinv_pad, blk_a, blk_b, off_a, off_b, gboff, idx_a, idx_b, dl_all


def _build_and_run(inputs_np, dinv_pad, blk_a, blk_b, off_a, off_b,
                   gboff, idx_a, idx_b, dl_all, trace=False, sim=False):
    import concourse.bacc as bacc
    import concourse.mybir as mybir
    from concourse.tile import TileContext
    from concourse import bass, bass_utils, library_config

    x = inputs_np["x"]
    Ws = [np.asarray(inputs_np[k], np.float32) for k in ("W1", "W2", "W3")]
    bs = [np.asarray(inputs_np[k], np.float32) for k in ("b1", "b2", "b3")]
    # pad W3/b3 to 128 output features
    W3p = np.zeros((F, F), np.float32); W3p[:, :F_OUT] = Ws[2]
    b3p = np.zeros(F, np.float32); b3p[:F_OUT] = bs[2]
    Ws[2], bs[2] = W3p, b3p
    b_nonzero = [bool(np.any(b)) for b in bs]

    n_a = idx_a.shape[1]
    n_b = idx_b.shape[1] if np.any(blk_b) else 0
    tot_blk = int(gboff[-1])
    maxblk = int((blk_a + blk_b).max())
    # gather groups of GROUP_WINDOWS windows
    groups = [list(range(g, min(g + GROUP_WINDOWS, N_WIN)))
              for g in range(0, N_WIN, GROUP_WINDOWS)]
    ga = [(int(off_a[g[0]]), int(off_a[g[-1] + 1])) for g in groups]
    gb_ = [(int(off_b[g[0]]), int(off_b[g[-1] + 1])) for g in groups]
    cap_a = max(b - a for a, b in ga) // 128
    cap_b = max(1, max(b - a for a, b in gb_) // 128)

    nc = bacc.Bacc("TRN2", target_bir_lowering=False, debug=False,
                   num_devices=N_CORES, num_swdge_queues=4)
    dt = mybir.dt

    # ---- kernel I/O -----------------------------------------------------
    t_xT = nc.dram_tensor("xT_own", [128, PER_CORE], dt.bfloat16, kind="ExternalInput")
    t_W = [nc.dram_tensor(f"W{i+1}m", [F, F], dt.bfloat16, kind="ExternalInput") for i in range(3)]
    t_bc = [nc.dram_tensor(f"b{i+1}c", [128, 1], dt.float32, kind="ExternalInput") for i in range(3)]
    t_b3 = nc.dram_tensor("b3m", [128, F], dt.float32, kind="ExternalInput")
    t_dinv = nc.dram_tensor("dinv_own", [128, N_WIN], dt.float32, kind="ExternalInput")
    t_dinvbc = nc.dram_tensor("dinv_bc", [128, PER_CORE], dt.float32, kind="ExternalInput")
    t_iota = nc.dram_tensor("iota", [128, 128], dt.bfloat16, kind="ExternalInput")
    t_identb = nc.dram_tensor("identb", [128, 128], dt.bfloat16, kind="ExternalInput")
    t_ia = nc.dram_tensor("idx_a", [128, n_a // 16], dt.int16, kind="ExternalInput")
    t_ib = nc.dram_tensor("idx_b", [128, max(1, n_b) // 16 if n_b else 1], dt.int16, kind="ExternalInput")
    t_dl = nc.dram_tensor("dl_all", [128, tot_blk], dt.float32, kind="ExternalInput")
    t_out = nc.dram_tensor("h_out", [PER_CORE, F_OUT], dt.float32, kind="ExternalOutput")

    with TileContext(nc) as tc:
        nc.gpsimd.load_library(library_config.mlp)
        with tc.tile_pool(name="const", bufs=1) as cpool, \
             tc.tile_pool(name="state", bufs=1) as spool, \
             tc.tile_pool(name="gA", bufs=PREFETCH_A + 1) as gpoolA, \
             tc.tile_pool(name="gB", bufs=PREFETCH_B + 2) as gpoolB, \
             tc.tile_pool(name="spool", bufs=4) as spoolS, \
             tc.tile_pool(name="psA", bufs=2, space="PSUM") as psA, \
             tc.tile_pool(name="psB", bufs=4, space="PSUM") as psB, \
             tc.tile_pool(name="dram", bufs=1, space="DRAM") as dpool:

            # ---- constants ----
            c_W = [cpool.tile([F, F], dt.bfloat16, tag=f"W{i}", name=f"cW{i}") for i in range(3)]
            c_bc = [cpool.tile([128, 1], dt.float32, tag=f"bc{i}", name=f"cbc{i}") for i in range(3)]
            c_b3 = cpool.tile([128, F], dt.float32, tag="b3", name="cb3")
            c_dinv = cpool.tile([128, N_WIN], dt.float32, tag="dinv", name="dinv")
            c_dinvbc = cpool.tile([128, N_WIN, 128], dt.float32, tag="dinvbc", name="dinvbc")
            c_iota = cpool.tile([128, 128], dt.bfloat16, tag="iota", name="iota")
            c_identb = cpool.tile([128, 128], dt.bfloat16, tag="identb", name="identb")
            c_ia = cpool.tile([128, n_a // 16], dt.int16, tag="ia", name="ia")
            c_ib = cpool.tile([128, max(1, n_b) // 16 if n_b else 1], dt.int16, tag="ib", name="ib")
            c_dl = cpool.tile([128, tot_blk], dt.float32, tag="dl", name="dl")
            # phase-A-critical consts first so layer 0 starts promptly;
            # the heavy gather tables stream in behind them.
            for i in range(3):
                nc.sync.dma_start(c_W[i][:], t_W[i][:])
                nc.sync.dma_start(c_bc[i][:], t_bc[i][:])
            nc.sync.dma_start(c_dinv[:], t_dinv[:])
            nc.sync.dma_start(c_b3[:], t_b3[:])
            nc.sync.dma_start(c_iota[:], t_iota[:])
            nc.sync.dma_start(c_identb[:], t_identb[:])

            # ---- persistent state ----
            hT = [spool.tile([128, PER_CORE], dt.bfloat16, tag="hT_a", name="hT_a"),
                  spool.tile([128, PER_CORE], dt.bfloat16, tag="hT_b", name="hT_b")]
            nc.sync.dma_start(hT[0][:], t_xT[:])
            nc.sync.dma_start(c_ia[:], t_ia[:])
            nc.sync.dma_start(c_ib[:], t_ib[:])
            nc.sync.dma_start(c_dl[:], t_dl[:])
            nc.sync.dma_start(
                c_dinvbc[:].rearrange("p t f -> p (t f)"), t_dinvbc[:])
            y_sbs = [spool.tile([128, N_WIN, F], dt.bfloat16, tag=f"y_sb{i}",
                                name=f"y_sb{i}") for i in range(2)]
            out_sb = spool.tile([128, N_WIN, F_OUT], dt.float32, tag="out_sb", name="out_sb")

            y_fa = [dpool.tile([ROWS_A, F], dt.bfloat16, addr_space="Shared",
                               name=f"y_fa{i}") for i in range(3)]
            y_fb = [dpool.tile([ROWS_B, F], dt.bfloat16, addr_space="Shared",
                               name=f"y_fb{i}") for i in range(3)]
            ag_a = [dpool.tile([LOC_A, F], dt.bfloat16, name=f"ag_a{i}")
                    for i in range(3)]
            ag_b = [dpool.tile([LOC_B, F], dt.bfloat16, name=f"ag_b{i}")
                    for i in range(3)]

            def phase_a_step(layer, t):
                """y(layer) for window t = dinv * (h @ W); fire the half-table
                AllGathers as soon as their windows are done."""
                y_sb = y_sbs[layer % 2]
                ps = psA.tile([128, F], dt.float32, tag="psA", space="PSUM")
                nc.tensor.matmul(ps[:], lhsT=hT[layer % 2][:, t * 128:(t + 1) * 128],
                                 rhs=c_W[layer][:], start=True, stop=True)
                nc.vector.tensor_scalar(
                    out=y_sb[:, t, :], in0=ps[:],
                    scalar1=c_dinv[:, t:t + 1], scalar2=None,
                    op0=mybir.AluOpType.mult)
                if t == WIN_A - 1:
                    nc.sync.dma_start(
                        ag_a[layer][:].rearrange("(t p) f -> p t f", p=128),
                        y_sb[:, :WIN_A, :])
                    nc.gpsimd.collective_compute(
                        "AllGather", mybir.AluOpType.bypass,
                        replica_groups=[list(range(N_CORES))],
                        ins=[ag_a[layer].opt()], outs=[y_fa[layer].opt()])
                if t == N_WIN - 1:
                    nc.sync.dma_start(
                        ag_b[layer][:].rearrange("(t p) f -> p t f", p=128),
                        y_sb[:, WIN_A:, :])
                    nc.gpsimd.collective_compute(
                        "AllGather", mybir.AluOpType.bypass,
                        replica_groups=[list(range(N_CORES))],
                        ins=[ag_b[layer].opt()], outs=[y_fb[layer].opt()])

            for t in range(N_WIN):
                phase_a_step(0, t)

            for layer in range(3):
                h_out = hT[(layer + 1) % 2]
                y_sb = y_sbs[layer % 2]
                # ---- phase B: gather + one-hot matmul aggregation ----
                qload = [0] * 4

                def pick_q(n):
                    q = min(range(4), key=lambda i: qload[i])
                    qload[q] += n
                    return q

                m_tiles = {}

                def emit_gather(gi, half):
                    a0, a1 = (ga if half == 0 else gb_)[gi]
                    pool = gpoolA if half == 0 else gpoolB
                    cap = cap_a if half == 0 else cap_b
                    tag = "ma" if half == 0 else "mb"
                    tile = pool.tile([128, cap, F], dt.bfloat16, tag=tag,
                                     name=tag)
                    m_tiles[(gi, half)] = tile
                    nb = (a1 - a0) // 128
                    if nb == 0:
                        return
                    src = y_fa[layer] if half == 0 else y_fb[layer]
                    idxs = c_ia if half == 0 else c_ib
                    mid = a0 + (nb - nb // 2) * 128
                    for (aa, bb) in ((a0, mid), (mid, a1)):
                        nn = bb - aa
                        if nn == 0:
                            continue
                        nc.gpsimd.dma_gather(
                            out_ap=tile[:, (aa - a0) // 128:(bb - a0) // 128, :],
                            in_ap=src[:],
                            idxs_ap=idxs[:, aa // 16:bb // 16],
                            num_idxs=nn, num_idxs_reg=nn, elem_size=F,
                            queue_num=pick_q(nn), single_packet=False)

                for gi in range(min(PREFETCH_A, len(groups))):
                    emit_gather(gi, 0)
                for gi in range(min(PREFETCH_B, len(groups))):
                    emit_gather(gi, 1)
                for gi, g in enumerate(groups):
                    if gi + PREFETCH_B < len(groups):
                        emit_gather(gi + PREFETCH_B, 1)
                    if gi + PREFETCH_A < len(groups):
                        emit_gather(gi + PREFETCH_A, 0)
                    m_a = m_tiles.pop((gi, 0))
                    m_b = m_tiles.pop((gi, 1))
                    a_base = ga[gi][0] // 128
                    b_base = gb_[gi][0] // 128
                    for w in g:
                        ba = int(blk_a[w]); bb = int(blk_b[w])
                        nblk = ba + bb
                        gbo = int(gboff[w])
                        # one-hot S for the whole window in one wide op
                        S = spoolS.tile([128, maxblk, 128], dt.bfloat16,
                                        tag="S", name="S")
                        nc.vector.tensor_tensor(
                            out=S[:, :nblk, :],
                            in0=c_iota[:].unsqueeze(1).broadcast_to([128, nblk, 128]),
                            in1=c_dl[:, gbo:gbo + nblk].unsqueeze(2)
                                .broadcast_to([128, nblk, 128]),
                            op=mybir.AluOpType.is_equal)

                        def m_ap(j):
                            if j < ba:
                                return m_a[:, int(off_a[w]) // 128 - a_base + j, :]
                            return m_b[:, int(off_b[w]) // 128 - b_base + (j - ba), :]

                        agg = psB.tile([128, F], dt.float32, tag="agg", space="PSUM")
                        if layer < 2:
                            # aggT[f, d]: self-loop y^T then messages
                            nc.tensor.matmul(agg[:], lhsT=y_sb[:, w, :],
                                             rhs=c_identb[:],
                                             start=True, stop=(nblk == 0))
                            for j in range(nblk):
                                nc.tensor.matmul(agg[:], lhsT=m_ap(j),
                                                 rhs=S[:, j, :],
                                                 start=False, stop=(j == nblk - 1))
                            # epilogue: h = dinv_col * aggT (+ b)
                            if b_nonzero[layer]:
                                hs = spoolS.tile([128, F], dt.float32, tag="hs", name="hs")
                                nc.vector.tensor_tensor(
                                    out=hs[:], in0=agg[:], in1=c_dinvbc[:, w, :],
                                    op=mybir.AluOpType.mult)
                                nc.vector.tensor_scalar(
                                    out=h_out[:, w * 128:(w + 1) * 128], in0=hs[:],
                                    scalar1=c_bc[layer][:], scalar2=None,
                                    op0=mybir.AluOpType.add)
                            else:
                                nc.vector.tensor_tensor(
                                    out=h_out[:, w * 128:(w + 1) * 128],
                                    in0=agg[:], in1=c_dinvbc[:, w, :],
                                    op=mybir.AluOpType.mult)
                            # pipeline: next layer's phase A for this window
                            phase_a_step(layer + 1, w)
                        else:
                            # agg[d, f]: self-loop then messages (dst-major)
                            nc.tensor.matmul(agg[:], lhsT=c_identb[:],
                                             rhs=y_sb[:, w, :],
                                             start=True, stop=(nblk == 0))
                            for j in range(nblk):
                                nc.tensor.matmul(agg[:], lhsT=S[:, j, :],
                                                 rhs=m_ap(j),
                                                 start=False, stop=(j == nblk - 1))
                            if b_nonzero[2]:
                                hs = spoolS.tile([128, F_OUT], dt.float32, tag="hs3", name="hs3")
                                nc.vector.scalar_tensor_tensor(
                                    out=hs[:], in0=agg[:, :F_OUT],
                                    scalar=c_dinv[:, w:w + 1], in1=c_b3[:, :F_OUT],
                                    op0=mybir.AluOpType.mult,
                                    op1=mybir.AluOpType.add)
                                nc.vector.tensor_scalar(
                                    out=out_sb[:, w, :], in0=hs[:],
                                    scalar1=0.0, scalar2=None,
                                    op0=mybir.AluOpType.max)
                            else:
                                # fused: relu(dinv * agg)
                                nc.vector.tensor_scalar(
                                    out=out_sb[:, w, :], in0=agg[:, :F_OUT],
                                    scalar1=c_dinv[:, w:w + 1], scalar2=0.0,
                                    op0=mybir.AluOpType.mult,
                                    op1=mybir.AluOpType.max)
            nc.sync.dma_start(
                t_out[:].rearrange("(t p) f -> p t f", p=128), out_sb[:])

    nc.compile()

    # ---- per-core inputs ----
    xT_all = np.zeros((128, N_PAD), np.float32)
    xT_all[:, :N_NODES] = np.asarray(x, np.float32).T
    iota_m = np.broadcast_to(np.arange(128, dtype=np.float32), (128, 128)).astype(BF16)
    ident_m = np.eye(128, dtype=np.float32).astype(BF16)
    in_maps = []
    for c in range(N_CORES):
        rows = slice(c * PER_CORE, (c + 1) * PER_CORE)
        din = dinv_pad[rows].reshape(N_WIN, 128).T.copy()  # [128, N_WIN]
        dinbc = np.broadcast_to(
            dinv_pad[rows].reshape(1, PER_CORE), (128, PER_CORE)).copy()
        in_map = {
            "xT_own": np.ascontiguousarray(xT_all[:, rows]).astype(BF16),
            "dinv_own": din,
            "dinv_bc": dinbc,
            "iota": iota_m.copy(),
            "identb": ident_m.copy(),
            "idx_a": _wrap_idx16(idx_a[c]),
            "idx_b": _wrap_idx16(idx_b[c]) if n_b else np.zeros((128, 1), np.int16),
            "dl_all": dl_all[c].reshape(-1, 128).T.copy(),
        }
        for i in range(3):
            in_map[f"W{i+1}m"] = Ws[i].astype(BF16)
            in_map[f"b{i+1}c"] = bs[i][:128].reshape(128, 1).astype(np.float32)
        in_map["b3m"] = np.broadcast_to(bs[2], (128, F)).astype(np.float32)
        in_maps.append(in_map)

    if sim:
        from concourse.bass_interp import MultiCoreSim
        mcs = MultiCoreSim(nc, num_cores=N_CORES, trace=False,
                           require_finite=False, require_nnan=False)
        for ci, core in enumerate(mcs.cores.values()):
            for k, v in in_maps[ci].items():
                core.tensor(k)[:] = v
        mcs.simulate(check_with_hw=False)
        outs = [np.asarray(core.tensor("h_out"))
                for core in mcs.cores.values()]
        res = None
    else:
        res = bass_utils.run_bass_kernel_spmd(
            nc, in_maps, core_ids=list(range(N_CORES)), trace=trace)
        outs = [r["h_out"] for r in res.results]
    full = np.concatenate(outs, axis=0)[:N_NODES]
    return full, res


def kernel(**inputs) -> np.ndarray:
    edge_index = np.asarray(inputs["edge_index"])
    prep = _preprocess(edge_index)
    out, _ = _build_and_run(inputs, *prep)
    return out
